# revision 17
# baseline (speedup 1.0000x reference)
"""Trainium2 Bass kernel for nn_DictSeparateActorResnBinary.

Data-parallel over 8 NeuronCores: batch B=128 -> 16 per core.

Per-core pipeline (all shapes hardcoded):
  1. 1x1 "conv": lin[c, hw] = merger_w[:, :131] @ concat(x, prediction)[b]
     via two accumulating f32r matmuls (K=128 x-channels, K=3 prediction).
  2. Padded grid aug (39x40 + float-faithful in_bounds variants B/C/D for
     coords == 21) with +w_ib folded into the interior.
  3. Per-unit dynamic-window gather + per-unit bias + leaky_relu fused:
     ACT path = one Lrelu activation, DVE path = ts-add + scalar_tensor_tensor.
  4. Heads: sap = sap_w.T @ merged (M=1 f32r matmuls, N=450);
     move = sum_p w3t[:, p, :].T @ merged[:, :, p] (225 accum matmuls, N=64).
  5. Masking (allowed-action mask with all-false -> all-true) + top-1 argmax
     on device via max_with_indices.
"""
import numpy as np

import concourse.bass as bass
import concourse.bacc as bacc
import concourse.mybir as mybir
from concourse.tile import TileContext
from concourse.tile_rust import add_dep_helper
from concourse.bass_utils import run_bass_kernel_spmd

F32 = mybir.dt.float32
F32R = mybir.dt.float32r
I32 = mybir.dt.int32
U32 = mybir.dt.uint32
U8 = mybir.dt.uint8
AL = mybir.AluOpType
AF = mybir.ActivationFunctionType

NCORES = 8
B, C, H, W = 128, 128, 24, 24
BL = B // NCORES          # 16 batch items per core
M = 16                    # units per batch item
N = 15                    # patch side
P = N * N                 # 225
NEMB, DEMB = 103, 32
UNITS = BL * M            # 256 units per core
GW = 40                   # padded grid row stride
GROWS = 107               # 39 (A) + 15 (B) + 38 (C) + 15 (D)
GF = GROWS * GW           # 4280 flat grid size
OFF_B, OFF_C, OFF_D = 39 * GW, 54 * GW, 92 * GW
NQ = 4                    # quarters
QB = BL // NQ             # 4 batch items per quarter
QU = QB * M               # 64 units per quarter
ACT_PER_B = 15            # units per batch item on the ACT path (rest on DVE)

_CACHE = {}


def _build():
    nc = bacc.Bacc()

    # ---- dram I/O ----
    d_xs = nc.dram_tensor("xs", [BL, C, H * W], F32R, kind="ExternalInput")
    d_ps = nc.dram_tensor("ps", [BL, 3, H * W], F32R, kind="ExternalInput")
    d_offs = nc.dram_tensor("offs", [1, UNITS], I32, kind="ExternalInput")
    d_embf = nc.dram_tensor("embf", [1, UNITS * 4], F32, kind="ExternalInput")
    d_cont = nc.dram_tensor("cont", [5, UNITS], F32R, kind="ExternalInput")
    d_aamw = nc.dram_tensor("aamw", [UNITS, 6], F32, kind="ExternalInput")
    d_aams = nc.dram_tensor("aams", [UNITS, P], F32, kind="ExternalInput")
    d_wxt = nc.dram_tensor("wxt", [C, C], F32R, kind="ExternalInput")
    d_wpt = nc.dram_tensor("wpt", [3, C], F32R, kind="ExternalInput")
    d_wet = nc.dram_tensor("wet", [C, C], F32R, kind="ExternalInput")
    d_wct = nc.dram_tensor("wct", [5, C], F32R, kind="ExternalInput")
    d_emtab = nc.dram_tensor("emtab", [NEMB, DEMB], F32R, kind="ExternalInput")
    d_sapw = nc.dram_tensor("sapw", [C, 1], F32R, kind="ExternalInput")
    d_w3t = nc.dram_tensor("w3t", [C, P * 6], F32R, kind="ExternalInput")
    d_wib = nc.dram_tensor("wib", [C, 1], F32, kind="ExternalInput")
    d_mgb = nc.dram_tensor("mgb", [C, 1], F32, kind="ExternalInput")
    d_sapb = nc.dram_tensor("sapb", [32, 1], F32, kind="ExternalInput")
    d_mvb = nc.dram_tensor("mvb", [6, 1], F32, kind="ExternalInput")
    d_id6 = nc.dram_tensor("id6", [6, 6], F32, kind="ExternalInput")

    o_wl = nc.dram_tensor("o_wl", [UNITS, 6], F32, kind="ExternalOutput")
    o_sl = nc.dram_tensor("o_sl", [UNITS, P], F32, kind="ExternalOutput")
    o_wa = nc.dram_tensor("o_wa", [UNITS, 1], F32, kind="ExternalOutput")
    o_sa = nc.dram_tensor("o_sa", [UNITS, 1], F32, kind="ExternalOutput")

    with TileContext(nc) as tc:
        with (
            tc.tile_pool(name="const", bufs=1) as cpool,
            tc.tile_pool(name="persist", bufs=1) as perm,
            tc.tile_pool(name="psc", bufs=2, space="PSUM") as psc,
        ):
            wxt = cpool.tile([C, C], F32R)
            wpt = cpool.tile([3, C], F32R)
            sapw = cpool.tile([C, 1], F32R)
            w3t = cpool.tile([C, P, 6], F32R)
            wib = cpool.tile([C, 1], F32)
            mgb = cpool.tile([C, 1], F32)
            sapb = cpool.tile([32, 1], F32)
            mvb = cpool.tile([6, 1], F32)
            id6 = cpool.tile([6, 6], F32)
            offs = cpool.tile([1, UNITS], I32)
            nc.sync.dma_start(wxt[:], d_wxt[:])
            nc.sync.dma_start(wpt[:], d_wpt[:])
            nc.sync.dma_start(sapw[:], d_sapw[:])
            nc.sync.dma_start(w3t[:], d_w3t[:].rearrange("c (p k) -> c p k", k=6))
            nc.sync.dma_start(wib[:], d_wib[:])
            nc.sync.dma_start(mgb[:], d_mgb[:])
            nc.sync.dma_start(sapb[:], d_sapb[:])
            nc.sync.dma_start(mvb[:], d_mvb[:])
            nc.sync.dma_start(id6[:], d_id6[:])
            nc.sync.dma_start(offs[:], d_offs[:])

            ub = perm.tile([C, UNITS], F32)
            sap_all0 = perm.tile([128, P], F32)
            sap_all1 = perm.tile([128, P], F32)
            wl_all0 = perm.tile([128, 6], F32)
            wl_all1 = perm.tile([128, 6], F32)
            sap_all = (sap_all0, sap_all1)
            wl_all = (wl_all0, wl_all1)
            head_writes = {0: [], 1: []}

            # ---- per-unit bias ub = Wu @ [emb; cont] + merger_b ----
            with (
                tc.tile_pool(name="embp", bufs=1) as ep,
                tc.tile_pool(name="pse", bufs=2, space="PSUM") as pse,
            ):
                wet = ep.tile([C, C], F32R)
                wct = ep.tile([5, C], F32R)
                emtab = ep.tile([NEMB, DEMB], F32R)
                idxf = ep.tile([1, UNITS * 4], F32)
                cont = ep.tile([5, UNITS], F32R)
                nc.sync.dma_start(wet[:], d_wet[:])
                nc.sync.dma_start(wct[:], d_wct[:])
                nc.sync.dma_start(emtab[:], d_emtab[:])
                nc.sync.dma_start(idxf[:], d_embf[:])
                nc.sync.dma_start(cont[:], d_cont[:])
                with tc.tile_pool(name="dsc", bufs=1, space="DRAM") as dscp:
                    d_sc = dscp.tile([1, UNITS * 4], F32)
                    nc.sync.dma_start(d_sc[:], idxf[:])
                    idxrep = ep.tile([NEMB, UNITS * 4], F32)
                    nc.sync.dma_start(idxrep[:], d_sc[:].to_broadcast([NEMB, UNITS * 4]))
                iocol = ep.tile([NEMB, 1], F32)
                nc.gpsimd.iota(iocol[:], pattern=[[0, 1]], base=0,
                               channel_multiplier=1,
                               allow_small_or_imprecise_dtypes=True)
                oneh = ep.tile([NEMB, UNITS * 4], F32R)
                nc.vector.tensor_scalar(oneh[:], idxrep[:], iocol[:, 0:1], None,
                                        op0=AL.is_equal)
                emb_sb = ep.tile([C, UNITS], F32R)
                for s in range(4):
                    pe_ = pse.tile([DEMB, UNITS], F32)
                    nc.tensor.matmul(pe_[:], emtab[:],
                                     oneh[:, UNITS * s:UNITS * (s + 1)],
                                     start=True, stop=True)
                    nc.vector.tensor_copy(emb_sb[DEMB * s:DEMB * (s + 1), :], pe_[:])
                pu = pse.tile([C, UNITS], F32)
                nc.tensor.matmul(pu[:], wet[:], emb_sb[:], start=True, stop=False)
                nc.tensor.matmul(pu[:], wct[:], cont[:], start=False, stop=True)
                nc.vector.tensor_scalar(ub[:], pu[:], mgb[:, 0:1], None, op0=AL.add)

            # ---- main loop ----
            with (
                tc.tile_pool(name="xsp", bufs=2) as xsp,
                tc.tile_pool(name="augp", bufs=2) as augp,
                tc.tile_pool(name="mrg", bufs=2) as mrgp,
                tc.tile_pool(name="tsc", bufs=2) as tscp,
                tc.tile_pool(name="sevac", bufs=2) as sevp,
                tc.tile_pool(name="psq", bufs=3, space="PSUM") as psq,
                tc.tile_pool(name="psmv", bufs=2, space="PSUM") as psmv,
                tc.tile_pool(name="psmt", bufs=1, space="PSUM") as psmt,
            ):
                regs_v = [nc.vector.alloc_register(f"rgv{i}") for i in range(M)]
                regs_a = [nc.scalar.alloc_register(f"rga{i}") for i in range(M)]
                merged = None
                for b in range(BL):
                    q, bq = b // QB, b % QB
                    if bq == 0:
                        merged = mrgp.tile([C, QU, N, 16], F32R)
                    xs_b = xsp.tile([C, 576], F32R, tag="xs")
                    ps_b = xsp.tile([3, 576], F32R, tag="ps")
                    nc.sync.dma_start(xs_b[:], d_xs[b])
                    nc.sync.dma_start(ps_b[:], d_ps[b])
                    aug = augp.tile([C, GROWS, GW], F32)
                    if b < 2:
                        nc.vector.memset(aug[:], 0.0)
                    for j in range(2):
                        pc = psc.tile([C, 288], F32)
                        nc.tensor.matmul(pc[:], wxt[:], xs_b[:, 288 * j:288 * (j + 1)],
                                         start=True, stop=False)
                        nc.tensor.matmul(pc[:], wpt[:], ps_b[:, 288 * j:288 * (j + 1)],
                                         start=False, stop=True)
                        nc.vector.tensor_scalar(
                            aug[:, 7 + 12 * j:19 + 12 * j, 7:31],
                            pc[:].rearrange("c (r w) -> c r w", w=24),
                            wib[:, 0:1], None, op0=AL.add)
                    # float-faithful in_bounds variants (coord == 21 anomaly)
                    augf = aug[:].rearrange("c r w -> c (r w)")
                    nc.sync.dma_start(augf[:, OFF_B:OFF_B + 600], augf[:, 840:1440])
                    nc.sync.dma_start(aug[:, 54:92, 0:16], aug[:, 0:38, 21:37])
                    nc.sync.dma_start(aug[:, 92:107, 0:16], aug[:, 21:36, 21:37])
                    nc.vector.tensor_scalar(aug[:, 48, 7:31], aug[:, 48, 7:31],
                                            wib[:, 0:1], None, op0=AL.subtract)
                    nc.vector.tensor_scalar(aug[:, 61:85, 9:10], aug[:, 61:85, 9:10],
                                            wib[:, 0:1], None, op0=AL.subtract)
                    nc.vector.tensor_scalar(aug[:, 101, 0:10], aug[:, 101, 0:10],
                                            wib[:, 0:1], None, op0=AL.subtract)
                    nc.vector.tensor_scalar(aug[:, 92:101, 9:10], aug[:, 92:101, 9:10],
                                            wib[:, 0:1], None, op0=AL.subtract)
                    # per-unit gather + bias + leaky
                    nc.vector.reg_load(regs_v, offs[0:1, M * b:M * (b + 1)])
                    nc.scalar.reg_load(regs_a, offs[0:1, M * b:M * (b + 1)])
                    for m in range(M):
                        bm = M * b + m
                        u = M * bq + m
                        if m < ACT_PER_B:
                            offv = nc.scalar.snap(regs_a[m], min_val=0, max_val=3680)
                            win = aug[:].rearrange("c r w -> c (r w)")[
                                :, bass.ds(offv, 600)].rearrange(
                                "c (r w) -> c r w", w=GW)[:, :N, :16]
                            nc.scalar.activation(merged[:, u], win, AF.Lrelu,
                                                 bias=ub[:, bm:bm + 1], scale=1.0,
                                                 alpha=0.01)
                        else:
                            offv = nc.vector.snap(regs_v[m], min_val=0, max_val=3680)
                            win = aug[:].rearrange("c r w -> c (r w)")[
                                :, bass.ds(offv, 600)].rearrange(
                                "c (r w) -> c r w", w=GW)[:, :N, :16]
                            t = tscp.tile([C, N, 16], F32, tag="t")
                            nc.vector.tensor_scalar(t[:], win, ub[:, bm:bm + 1],
                                                    None, op0=AL.add)
                            nc.vector.scalar_tensor_tensor(
                                merged[:, u], t[:], 0.01, t[:],
                                op0=AL.mult, op1=AL.max)
                    if bq == QB - 1:
                        # ---- heads for this quarter ----
                        chunk = (QU * q) // 128
                        r0 = (QU * q) % 128
                        for jj in range(32):
                            p_sap = psq.tile([1, 480], F32, tag="psap")
                            nc.tensor.matmul(
                                p_sap[:], sapw[:],
                                merged[:, 2 * jj:2 * jj + 2],
                                start=True, stop=True)
                            sapstage = sevp.tile([1, 2, N, 16], F32, tag="sap")
                            nc.vector.tensor_scalar(
                                sapstage[:], p_sap[:],
                                sapb[0:1, 0:1], None, op0=AL.add)
                            _i = nc.sync.dma_start(
                                sap_all[chunk][r0 + 2 * jj:r0 + 2 * jj + 2, :],
                                sapstage[:, :, :, 0:N])
                            head_writes[chunk].append(_i)
                        p_mv = psmv.tile([6, QU], F32, tag="mv")
                        for pp in range(P):
                            i_, j_ = pp // N, pp % N
                            nc.tensor.matmul(
                                p_mv[:], w3t[:, pp, :],
                                merged[:, :, i_, j_],
                                start=(pp == 0), stop=(pp == P - 1))
                        mv_sb = sevp.tile([6, QU], F32, tag="mvs")
                        nc.vector.tensor_scalar(mv_sb[:], p_mv[:], mvb[:, 0:1],
                                                None, op0=AL.add)
                        p_mvt = psmt.tile([QU, 6], F32, tag="mvt")
                        nc.tensor.transpose(p_mvt[:], mv_sb[:], id6[:])
                        _i = nc.vector.tensor_copy(wl_all[chunk][r0:r0 + QU, :],
                                                   p_mvt[:])
                        head_writes[chunk].append(_i)

                for r in regs_v:
                    nc.vector.free_register(r)
                for r in regs_a:
                    nc.scalar.free_register(r)

            # ---- masking + argmax ----
            with tc.tile_pool(name="outp", bufs=1) as op_:
                neginf = op_.tile([128, P], F32)
                nc.vector.memset(neginf[:], float("-inf"))
                neg30 = op_.tile([128, P], F32)
                nc.vector.memset(neg30[:], -1e30)
                iota_t = op_.tile([128, P], F32)
                nc.gpsimd.iota(iota_t[:], pattern=[[1, P]], base=1000000,
                               channel_multiplier=0,
                               allow_small_or_imprecise_dtypes=True)
                for chunk in range(2):
                    r0 = 128 * chunk
                    awf = op_.tile([128, 6], F32, tag=f"awf{chunk}")
                    asf = op_.tile([128, P], F32, tag=f"asf{chunk}")
                    nc.sync.dma_start(awf[:], d_aamw[r0:r0 + 128, :])
                    nc.sync.dma_start(asf[:], d_aams[r0:r0 + 128, :])
                    for (logits, aam, nact, d_l, d_a) in (
                        (wl_all[chunk], awf, 6, o_wl, o_wa),
                        (sap_all[chunk], asf, P, o_sl, o_sa),
                    ):
                        anyv = op_.tile([128, 1], F32, tag=f"any{chunk}{nact}")
                        nc.vector.tensor_reduce(out=anyv[:], in_=aam[:],
                                                op=AL.max, axis=mybir.AxisListType.X)
                        onem = op_.tile([128, 1], F32, tag=f"onem{chunk}{nact}")
                        nc.vector.tensor_scalar(onem[:], anyv[:], -1.0, 1.0,
                                                op0=AL.mult, op1=AL.add)
                        filled = op_.tile([128, nact], F32, tag=f"fil{chunk}{nact}")
                        nc.vector.tensor_scalar(filled[:], aam[:], onem[:, 0:1],
                                                None, op0=AL.max)
                        fu8 = op_.tile([128, nact], U8, tag=f"fu8{chunk}{nact}")
                        nc.vector.tensor_copy(fu8[:], filled[:])
                        lm = op_.tile([128, max(nact, 8)], F32, tag=f"lm{chunk}{nact}")
                        if nact < 8:
                            nc.vector.memset(lm[:, nact:8], float("-inf"))
                        nc.vector.select(lm[:, 0:nact], fu8[:], logits[:],
                                         neginf[:, 0:nact])
                        la = op_.tile([128, nact], F32, tag=f"la{chunk}{nact}")
                        nc.vector.select(la[:], fu8[:], logits[:],
                                         neg30[:, 0:nact])
                        rmax = op_.tile([128, 1], F32, tag=f"rmax{chunk}{nact}")
                        nc.vector.tensor_reduce(out=rmax[:], in_=la[:],
                                                op=AL.max, axis=mybir.AxisListType.X)
                        eqm = op_.tile([128, nact], F32, tag=f"eqm{chunk}{nact}")
                        nc.vector.tensor_scalar(eqm[:], la[:], rmax[:, 0:1], None,
                                                op0=AL.is_equal)
                        cand = op_.tile([128, nact], F32, tag=f"cand{chunk}{nact}")
                        nc.vector.scalar_tensor_tensor(
                            cand[:], eqm[:], -1e6, iota_t[:, 0:nact],
                            op0=AL.mult, op1=AL.add)
                        amin = op_.tile([128, 1], F32, tag=f"amin{chunk}{nact}")
                        nc.vector.tensor_reduce(out=amin[:], in_=cand[:],
                                                op=AL.min, axis=mybir.AxisListType.X)
                        nc.sync.dma_start(d_l[r0:r0 + 128, :], lm[:, 0:nact])
                        nc.sync.dma_start(d_a[r0:r0 + 128, :], amin[:])

    nc.compile()
    return nc


def _host_prep(inputs):
    """Shard + lay out inputs for the 8 cores. Pure reshapes/index math."""
    x = np.ascontiguousarray(inputs["x"], dtype=np.float32)
    pred = np.ascontiguousarray(inputs["prediction"], dtype=np.float32)
    xc = np.asarray(inputs["x_cord"], dtype=np.int32)
    yc = np.asarray(inputs["y_cord"], dtype=np.int32)
    emf = np.asarray(inputs["embedding_features"], dtype=np.int32)
    conf = np.asarray(inputs["continues_features"], dtype=np.float32)
    aamw = np.asarray(inputs["aam_worker"])
    aams = np.asarray(inputs["aam_sapper"])
    mw = np.asarray(inputs["merger_w"], dtype=np.float32)
    mb = np.asarray(inputs["merger_b"], dtype=np.float32)
    sw = np.asarray(inputs["sap_w"], dtype=np.float32)
    sb_ = np.asarray(inputs["sap_b"], dtype=np.float32)
    mvw = np.asarray(inputs["move_w"], dtype=np.float32)
    mvb = np.asarray(inputs["move_b"], dtype=np.float32)
    emt = np.asarray(inputs["emb_table"], dtype=np.float32)

    consts = dict(
        wxt=np.ascontiguousarray(mw[:, 0:128].T),
        wpt=np.ascontiguousarray(mw[:, 128:131].T),
        wet=np.ascontiguousarray(mw[:, 132:260].T),
        wct=np.ascontiguousarray(mw[:, 260:265].T),
        emtab=np.ascontiguousarray(emt),
        sapw=np.ascontiguousarray(sw[0][:, None]),
        w3t=np.ascontiguousarray(
            mvw.reshape(6, P, C).transpose(2, 1, 0).reshape(C, P * 6)),
        wib=np.ascontiguousarray(mw[:, 131][:, None]),
        mgb=np.ascontiguousarray(mb[:, None]),
        sapb=np.full((32, 1), sb_[0], dtype=np.float32),
        mvb=np.ascontiguousarray(mvb[:, None]),
        id6=np.eye(6, dtype=np.float32),
    )

    # window offsets with in_bounds variant redirects (coord == 21)
    def offsets(xs_, ys_):
        offa = GW * xs_ + ys_
        ex = xs_ == 21
        ey = ys_ == 21
        off = np.where(ex & ey, OFF_D,
                       np.where(ex, OFF_B + ys_,
                                np.where(ey, OFF_C + GW * xs_, offa)))
        return off.astype(np.int32)

    in_maps = []
    for c in range(NCORES):
        sl = slice(BL * c, BL * (c + 1))
        xs_ = xc[sl].reshape(-1)
        ys_ = yc[sl].reshape(-1)
        in_maps.append(dict(
            xs=np.ascontiguousarray(x[sl].reshape(BL, C, H * W)),
            ps=np.ascontiguousarray(pred[sl].reshape(BL, 3, H * W)),
            offs=offsets(xs_, ys_)[None, :],
            embf=np.ascontiguousarray(
                emf[sl].reshape(UNITS, 4).T).reshape(1, -1).astype(np.float32),
            cont=np.ascontiguousarray(conf[sl].reshape(UNITS, 5).T),
            aamw=aamw.reshape(B, M, 6)[sl].reshape(UNITS, 6).astype(np.float32),
            aams=aams.reshape(B, M, P)[sl].reshape(UNITS, P).astype(np.float32),
            **consts,
        ))
    return in_maps


def kernel(**inputs):
    if "nc" not in _CACHE:
        _CACHE["nc"] = _build()
    nc = _CACHE["nc"]
    in_maps = _host_prep(inputs)
    res = run_bass_kernel_spmd(nc, in_maps, list(range(NCORES)),
                               **_CACHE.get("run_kwargs", {}))
    _CACHE["last_res"] = res
    wl = np.concatenate([r["o_wl"] for r in res.results], 0)   # (2048, 6)
    sl = np.concatenate([r["o_sl"] for r in res.results], 0)   # (2048, 225)
    wa = np.concatenate([r["o_wa"] for r in res.results], 0)
    sa = np.concatenate([r["o_sa"] for r in res.results], 0)
    wl = wl.reshape(B // 2, 2, M, 6)[:, None]
    sl = sl.reshape(B // 2, 2, M, P)[:, None]
    wa = wa.reshape(B // 2, 2, M, 1)[:, None].astype(np.int32)
    sa = sa.reshape(B // 2, 2, M, 1)[:, None].astype(np.int32)
    return wl, sl, wa, sa


# revision 19
# speedup vs baseline: 1.0769x; 1.0769x over previous
"""Trainium2 Bass kernel for nn_DictSeparateActorResnBinary.

Data-parallel over 8 NeuronCores: batch B=128 -> 16 per core.

Per-core pipeline (all shapes hardcoded):
  1. 1x1 "conv": lin[c, hw] = merger_w[:, :131] @ concat(x, prediction)[b]
     via two accumulating f32r matmuls (K=128 x-channels, K=3 prediction).
  2. Padded grid aug (39x40 + float-faithful in_bounds variants B/C/D for
     coords == 21) with +w_ib folded into the interior.
  3. Per-unit dynamic-window gather + per-unit bias + leaky_relu fused:
     ACT path = one Lrelu activation, DVE path = ts-add + scalar_tensor_tensor.
  4. Heads: sap = sap_w.T @ merged (M=1 f32r matmuls, N=450);
     move = sum_p w3t[:, p, :].T @ merged[:, :, p] (225 accum matmuls, N=64).
  5. Masking (allowed-action mask with all-false -> all-true) + top-1 argmax
     on device via max_with_indices.
"""
import numpy as np

import concourse.bass as bass
import concourse.bacc as bacc
import concourse.mybir as mybir
from concourse.tile import TileContext
from concourse.tile_rust import add_dep_helper
from concourse.bass_utils import run_bass_kernel_spmd

F32 = mybir.dt.float32
F32R = mybir.dt.float32r
I32 = mybir.dt.int32
U32 = mybir.dt.uint32
U8 = mybir.dt.uint8
AL = mybir.AluOpType
AF = mybir.ActivationFunctionType

NCORES = 8
B, C, H, W = 128, 128, 24, 24
BL = B // NCORES          # 16 batch items per core
M = 16                    # units per batch item
N = 15                    # patch side
P = N * N                 # 225
NEMB, DEMB = 103, 32
UNITS = BL * M            # 256 units per core
GW = 40                   # padded grid row stride
GROWS = 107               # 39 (A) + 15 (B) + 38 (C) + 15 (D)
GF = GROWS * GW           # 4280 flat grid size
OFF_B, OFF_C, OFF_D = 39 * GW, 54 * GW, 92 * GW
NQ = 4                    # quarters
QB = BL // NQ             # 4 batch items per quarter
QU = QB * M               # 64 units per quarter
ACT_PER_B = 15            # units per batch item on the ACT path (rest on DVE)

_CACHE = {}


def _build():
    nc = bacc.Bacc()

    # ---- dram I/O ----
    d_xs = nc.dram_tensor("xs", [BL, C, H * W], F32R, kind="ExternalInput")
    d_ps = nc.dram_tensor("ps", [BL, 3, H * W], F32R, kind="ExternalInput")
    d_offs = nc.dram_tensor("offs", [1, UNITS], I32, kind="ExternalInput")
    d_embf = nc.dram_tensor("embf", [1, UNITS * 4], F32, kind="ExternalInput")
    d_cont = nc.dram_tensor("cont", [5, UNITS], F32R, kind="ExternalInput")
    d_aamw = nc.dram_tensor("aamw", [UNITS, 6], F32, kind="ExternalInput")
    d_aams = nc.dram_tensor("aams", [UNITS, P], F32, kind="ExternalInput")
    d_wxt = nc.dram_tensor("wxt", [C, C], F32R, kind="ExternalInput")
    d_wpt = nc.dram_tensor("wpt", [3, C], F32R, kind="ExternalInput")
    d_wet = nc.dram_tensor("wet", [C, C], F32R, kind="ExternalInput")
    d_wct = nc.dram_tensor("wct", [5, C], F32R, kind="ExternalInput")
    d_emtab = nc.dram_tensor("emtab", [NEMB, DEMB], F32R, kind="ExternalInput")
    d_sapw = nc.dram_tensor("sapw", [C, 1], F32R, kind="ExternalInput")
    d_w3t = nc.dram_tensor("w3t", [C, P * 6], F32R, kind="ExternalInput")
    d_wib = nc.dram_tensor("wib", [C, 1], F32, kind="ExternalInput")
    d_mgb = nc.dram_tensor("mgb", [C, 1], F32, kind="ExternalInput")
    d_sapb = nc.dram_tensor("sapb", [32, 1], F32, kind="ExternalInput")
    d_mvb = nc.dram_tensor("mvb", [6, 1], F32, kind="ExternalInput")
    d_id6 = nc.dram_tensor("id6", [6, 6], F32, kind="ExternalInput")

    o_wl = nc.dram_tensor("o_wl", [UNITS, 6], F32, kind="ExternalOutput")
    o_sl = nc.dram_tensor("o_sl", [UNITS, P], F32, kind="ExternalOutput")
    o_wa = nc.dram_tensor("o_wa", [UNITS, 1], F32, kind="ExternalOutput")
    o_sa = nc.dram_tensor("o_sa", [UNITS, 1], F32, kind="ExternalOutput")

    with TileContext(nc) as tc:
        with (
            tc.tile_pool(name="const", bufs=1) as cpool,
            tc.tile_pool(name="persist", bufs=1) as perm,
            tc.tile_pool(name="psc", bufs=2, space="PSUM") as psc,
        ):
            wxt = cpool.tile([C, C], F32R)
            wpt = cpool.tile([3, C], F32R)
            sapw = cpool.tile([C, 1], F32R)
            w3t = cpool.tile([C, P, 6], F32R)
            wib = cpool.tile([C, 1], F32)
            mgb = cpool.tile([C, 1], F32)
            sapb = cpool.tile([32, 1], F32)
            mvb = cpool.tile([6, 1], F32)
            id6 = cpool.tile([6, 6], F32)
            offs = cpool.tile([1, UNITS], I32)
            nc.sync.dma_start(wxt[:], d_wxt[:])
            nc.sync.dma_start(wpt[:], d_wpt[:])
            nc.sync.dma_start(sapw[:], d_sapw[:])
            nc.sync.dma_start(w3t[:], d_w3t[:].rearrange("c (p k) -> c p k", k=6))
            nc.sync.dma_start(wib[:], d_wib[:])
            nc.sync.dma_start(mgb[:], d_mgb[:])
            nc.sync.dma_start(sapb[:], d_sapb[:])
            nc.sync.dma_start(mvb[:], d_mvb[:])
            nc.sync.dma_start(id6[:], d_id6[:])
            nc.sync.dma_start(offs[:], d_offs[:])

            ub = perm.tile([C, UNITS], F32)
            sap_all0 = perm.tile([128, P], F32)
            sap_all1 = perm.tile([128, P], F32)
            wl_all0 = perm.tile([128, 6], F32)
            wl_all1 = perm.tile([128, 6], F32)
            sap_all = (sap_all0, sap_all1)
            wl_all = (wl_all0, wl_all1)
            head_writes = {0: [], 1: []}

            # ---- per-unit bias ub = Wu @ [emb; cont] + merger_b ----
            with (
                tc.tile_pool(name="embp", bufs=1) as ep,
                tc.tile_pool(name="pse", bufs=2, space="PSUM") as pse,
            ):
                wet = ep.tile([C, C], F32R)
                wct = ep.tile([5, C], F32R)
                emtab = ep.tile([NEMB, DEMB], F32R)
                idxf = ep.tile([1, UNITS * 4], F32)
                cont = ep.tile([5, UNITS], F32R)
                nc.sync.dma_start(wet[:], d_wet[:])
                nc.sync.dma_start(wct[:], d_wct[:])
                nc.sync.dma_start(emtab[:], d_emtab[:])
                nc.sync.dma_start(idxf[:], d_embf[:])
                nc.sync.dma_start(cont[:], d_cont[:])
                with tc.tile_pool(name="dsc", bufs=1, space="DRAM") as dscp:
                    d_sc = dscp.tile([1, UNITS * 4], F32)
                    nc.sync.dma_start(d_sc[:], idxf[:])
                    idxrep = ep.tile([NEMB, UNITS * 4], F32)
                    nc.sync.dma_start(idxrep[:], d_sc[:].to_broadcast([NEMB, UNITS * 4]))
                iocol = ep.tile([NEMB, 1], F32)
                nc.gpsimd.iota(iocol[:], pattern=[[0, 1]], base=0,
                               channel_multiplier=1,
                               allow_small_or_imprecise_dtypes=True)
                oneh = ep.tile([NEMB, UNITS * 4], F32R)
                nc.vector.tensor_scalar(oneh[:], idxrep[:], iocol[:, 0:1], None,
                                        op0=AL.is_equal)
                emb_sb = ep.tile([C, UNITS], F32R)
                for s in range(4):
                    pe_ = pse.tile([DEMB, UNITS], F32)
                    nc.tensor.matmul(pe_[:], emtab[:],
                                     oneh[:, UNITS * s:UNITS * (s + 1)],
                                     start=True, stop=True)
                    nc.vector.tensor_copy(emb_sb[DEMB * s:DEMB * (s + 1), :], pe_[:])
                pu = pse.tile([C, UNITS], F32)
                nc.tensor.matmul(pu[:], wet[:], emb_sb[:], start=True, stop=False)
                nc.tensor.matmul(pu[:], wct[:], cont[:], start=False, stop=True)
                nc.vector.tensor_scalar(ub[:], pu[:], mgb[:, 0:1], None, op0=AL.add)

            # ---- main loop ----
            with (
                tc.tile_pool(name="xsp", bufs=2) as xsp,
                tc.tile_pool(name="augp", bufs=2) as augp,
                tc.tile_pool(name="mrg", bufs=2) as mrgp,
                tc.tile_pool(name="tsc", bufs=2) as tscp,
                tc.tile_pool(name="sevac", bufs=2) as sevp,
                tc.tile_pool(name="psq", bufs=2, space="PSUM") as psq,
                tc.tile_pool(name="psmv", bufs=1, space="PSUM") as psmv,
                tc.tile_pool(name="psmt", bufs=1, space="PSUM") as psmt,
            ):
                regs_v = [nc.vector.alloc_register(f"rgv{i}") for i in range(M)]
                regs_a = [nc.scalar.alloc_register(f"rga{i}") for i in range(M)]
                merged = None
                for b in range(BL):
                    q, bq = b // QB, b % QB
                    if bq == 0:
                        merged = mrgp.tile([C, QU, N, 16], F32R)
                    xs_b = xsp.tile([C, 576], F32R, tag="xs")
                    ps_b = xsp.tile([3, 576], F32R, tag="ps")
                    nc.sync.dma_start(xs_b[:], d_xs[b])
                    nc.sync.dma_start(ps_b[:], d_ps[b])
                    aug = augp.tile([C, GROWS, GW], F32)
                    if b < 2:
                        nc.vector.memset(aug[:], 0.0)
                    for j in range(2):
                        pc = psc.tile([C, 288], F32)
                        nc.tensor.matmul(pc[:], wxt[:], xs_b[:, 288 * j:288 * (j + 1)],
                                         start=True, stop=False)
                        nc.tensor.matmul(pc[:], wpt[:], ps_b[:, 288 * j:288 * (j + 1)],
                                         start=False, stop=True)
                        nc.vector.tensor_scalar(
                            aug[:, 7 + 12 * j:19 + 12 * j, 7:31],
                            pc[:].rearrange("c (r w) -> c r w", w=24),
                            wib[:, 0:1], None, op0=AL.add)
                    # float-faithful in_bounds variants (coord == 21 anomaly)
                    augf = aug[:].rearrange("c r w -> c (r w)")
                    nc.gpsimd.dma_start(augf[:, OFF_B:OFF_B + 600], augf[:, 840:1440])
                    nc.gpsimd.dma_start(aug[:, 54:92, 0:16], aug[:, 0:38, 21:37])
                    nc.gpsimd.dma_start(aug[:, 92:107, 0:16], aug[:, 21:36, 21:37])
                    nc.vector.tensor_scalar(aug[:, 48, 7:31], aug[:, 48, 7:31],
                                            wib[:, 0:1], None, op0=AL.subtract)
                    nc.vector.tensor_scalar(aug[:, 61:85, 9:10], aug[:, 61:85, 9:10],
                                            wib[:, 0:1], None, op0=AL.subtract)
                    nc.vector.tensor_scalar(aug[:, 101, 0:10], aug[:, 101, 0:10],
                                            wib[:, 0:1], None, op0=AL.subtract)
                    nc.vector.tensor_scalar(aug[:, 92:101, 9:10], aug[:, 92:101, 9:10],
                                            wib[:, 0:1], None, op0=AL.subtract)
                    # per-unit gather + bias + leaky
                    nc.scalar.reg_load(regs_a[:ACT_PER_B],
                                       offs[0:1, M * b:M * b + ACT_PER_B])
                    nc.vector.reg_load(regs_v[ACT_PER_B:],
                                       offs[0:1, M * b + ACT_PER_B:M * (b + 1)])
                    for m in range(M):
                        bm = M * b + m
                        u = M * bq + m
                        if m < ACT_PER_B:
                            offv = nc.scalar.snap(regs_a[m], min_val=0, max_val=3680)
                            win = aug[:].rearrange("c r w -> c (r w)")[
                                :, bass.ds(offv, 600)].rearrange(
                                "c (r w) -> c r w", w=GW)[:, :N, :16]
                            nc.scalar.activation(merged[:, u], win, AF.Lrelu,
                                                 bias=ub[:, bm:bm + 1], scale=1.0,
                                                 alpha=0.01)
                        else:
                            offv = nc.vector.snap(regs_v[m], min_val=0, max_val=3680)
                            win = aug[:].rearrange("c r w -> c (r w)")[
                                :, bass.ds(offv, 600)].rearrange(
                                "c (r w) -> c r w", w=GW)[:, :N, :16]
                            t = tscp.tile([C, N, 16], F32, tag="t")
                            nc.vector.tensor_scalar(t[:], win, ub[:, bm:bm + 1],
                                                    None, op0=AL.add)
                            nc.vector.scalar_tensor_tensor(
                                merged[:, u], t[:], 0.01, t[:],
                                op0=AL.mult, op1=AL.max)
                    if bq == QB - 1:
                        # ---- heads for this quarter ----
                        chunk = (QU * q) // 128
                        r0 = (QU * q) % 128
                        for g in range(8):
                            sapstage = sevp.tile([1, 4, 2, N, 16], F32, tag="sap")
                            for t in range(4):
                                jj = 4 * g + t
                                p_sap = psq.tile([1, 480], F32, tag="psap")
                                nc.tensor.matmul(
                                    p_sap[:], sapw[:],
                                    merged[:, 2 * jj:2 * jj + 2],
                                    start=True, stop=True)
                                if t % 2 == 0:
                                    nc.vector.tensor_scalar(
                                        sapstage[:, t], p_sap[:],
                                        sapb[0:1, 0:1], None, op0=AL.add)
                                else:
                                    nc.scalar.activation(
                                        sapstage[:, t], p_sap[:], AF.Identity,
                                        bias=sapb[0:1, 0:1], scale=1.0)
                            nc.sync.dma_start(
                                sap_all[chunk][r0 + 8 * g:r0 + 8 * g + 8, :],
                                sapstage[:, :, :, :, 0:N])
                        p_mv0 = psmv.tile([6, QU], F32, tag="mv0")
                        p_mv1 = psmv.tile([6, QU], F32, tag="mv1")
                        for pp in range(P):
                            i_, j_ = pp // N, pp % N
                            tgt = p_mv0 if pp % 2 == 0 else p_mv1
                            nc.tensor.matmul(
                                tgt[:], w3t[:, pp, :],
                                merged[:, :, i_, j_],
                                start=(pp < 2), stop=(pp >= P - 2))
                        mv_sb = sevp.tile([6, QU], F32, tag="mvs")
                        nc.vector.tensor_scalar(mv_sb[:], p_mv0[:], mvb[:, 0:1],
                                                None, op0=AL.add)
                        nc.vector.tensor_tensor(mv_sb[:], mv_sb[:], p_mv1[:],
                                                op=AL.add)
                        p_mvt = psmt.tile([QU, 6], F32, tag="mvt")
                        nc.tensor.transpose(p_mvt[:], mv_sb[:], id6[:])
                        nc.vector.tensor_copy(wl_all[chunk][r0:r0 + QU, :],
                                              p_mvt[:])

                for r in regs_v:
                    nc.vector.free_register(r)
                for r in regs_a:
                    nc.scalar.free_register(r)

            # ---- masking + argmax ----
            with tc.tile_pool(name="outp", bufs=1) as op_:
                neginf = op_.tile([128, P], F32)
                nc.vector.memset(neginf[:], float("-inf"))
                neg30 = op_.tile([128, P], F32)
                nc.vector.memset(neg30[:], -1e30)
                iota_t = op_.tile([128, P], F32)
                nc.gpsimd.iota(iota_t[:], pattern=[[1, P]], base=1000000,
                               channel_multiplier=0,
                               allow_small_or_imprecise_dtypes=True)
                for chunk in range(2):
                    r0 = 128 * chunk
                    awf = op_.tile([128, 6], F32, tag=f"awf{chunk}")
                    asf = op_.tile([128, P], F32, tag=f"asf{chunk}")
                    nc.sync.dma_start(awf[:], d_aamw[r0:r0 + 128, :])
                    nc.sync.dma_start(asf[:], d_aams[r0:r0 + 128, :])
                    for (logits, aam, nact, d_l, d_a) in (
                        (wl_all[chunk], awf, 6, o_wl, o_wa),
                        (sap_all[chunk], asf, P, o_sl, o_sa),
                    ):
                        anyv = op_.tile([128, 1], F32, tag=f"any{chunk}{nact}")
                        nc.vector.tensor_reduce(out=anyv[:], in_=aam[:],
                                                op=AL.max, axis=mybir.AxisListType.X)
                        onem = op_.tile([128, 1], F32, tag=f"onem{chunk}{nact}")
                        nc.vector.tensor_scalar(onem[:], anyv[:], -1.0, 1.0,
                                                op0=AL.mult, op1=AL.add)
                        filled = op_.tile([128, nact], F32, tag=f"fil{chunk}{nact}")
                        nc.vector.tensor_scalar(filled[:], aam[:], onem[:, 0:1],
                                                None, op0=AL.max)
                        fu8 = op_.tile([128, nact], U8, tag=f"fu8{chunk}{nact}")
                        nc.vector.tensor_copy(fu8[:], filled[:])
                        lm = op_.tile([128, max(nact, 8)], F32, tag=f"lm{chunk}{nact}")
                        if nact < 8:
                            nc.vector.memset(lm[:, nact:8], float("-inf"))
                        nc.vector.select(lm[:, 0:nact], fu8[:], logits[:],
                                         neginf[:, 0:nact])
                        la = op_.tile([128, nact], F32, tag=f"la{chunk}{nact}")
                        nc.vector.select(la[:], fu8[:], logits[:],
                                         neg30[:, 0:nact])
                        rmax = op_.tile([128, 1], F32, tag=f"rmax{chunk}{nact}")
                        nc.vector.tensor_reduce(out=rmax[:], in_=la[:],
                                                op=AL.max, axis=mybir.AxisListType.X)
                        eqm = op_.tile([128, nact], F32, tag=f"eqm{chunk}{nact}")
                        nc.vector.tensor_scalar(eqm[:], la[:], rmax[:, 0:1], None,
                                                op0=AL.is_equal)
                        cand = op_.tile([128, nact], F32, tag=f"cand{chunk}{nact}")
                        nc.vector.scalar_tensor_tensor(
                            cand[:], eqm[:], -1e6, iota_t[:, 0:nact],
                            op0=AL.mult, op1=AL.add)
                        amin = op_.tile([128, 1], F32, tag=f"amin{chunk}{nact}")
                        nc.vector.tensor_reduce(out=amin[:], in_=cand[:],
                                                op=AL.min, axis=mybir.AxisListType.X)
                        nc.sync.dma_start(d_l[r0:r0 + 128, :], lm[:, 0:nact])
                        nc.sync.dma_start(d_a[r0:r0 + 128, :], amin[:])

    nc.compile()
    return nc


def _host_prep(inputs):
    """Shard + lay out inputs for the 8 cores. Pure reshapes/index math."""
    x = np.ascontiguousarray(inputs["x"], dtype=np.float32)
    pred = np.ascontiguousarray(inputs["prediction"], dtype=np.float32)
    xc = np.asarray(inputs["x_cord"], dtype=np.int32)
    yc = np.asarray(inputs["y_cord"], dtype=np.int32)
    emf = np.asarray(inputs["embedding_features"], dtype=np.int32)
    conf = np.asarray(inputs["continues_features"], dtype=np.float32)
    aamw = np.asarray(inputs["aam_worker"])
    aams = np.asarray(inputs["aam_sapper"])
    mw = np.asarray(inputs["merger_w"], dtype=np.float32)
    mb = np.asarray(inputs["merger_b"], dtype=np.float32)
    sw = np.asarray(inputs["sap_w"], dtype=np.float32)
    sb_ = np.asarray(inputs["sap_b"], dtype=np.float32)
    mvw = np.asarray(inputs["move_w"], dtype=np.float32)
    mvb = np.asarray(inputs["move_b"], dtype=np.float32)
    emt = np.asarray(inputs["emb_table"], dtype=np.float32)

    consts = dict(
        wxt=np.ascontiguousarray(mw[:, 0:128].T),
        wpt=np.ascontiguousarray(mw[:, 128:131].T),
        wet=np.ascontiguousarray(mw[:, 132:260].T),
        wct=np.ascontiguousarray(mw[:, 260:265].T),
        emtab=np.ascontiguousarray(emt),
        sapw=np.ascontiguousarray(sw[0][:, None]),
        w3t=np.ascontiguousarray(
            mvw.reshape(6, P, C).transpose(2, 1, 0).reshape(C, P * 6)),
        wib=np.ascontiguousarray(mw[:, 131][:, None]),
        mgb=np.ascontiguousarray(mb[:, None]),
        sapb=np.full((32, 1), sb_[0], dtype=np.float32),
        mvb=np.ascontiguousarray(mvb[:, None]),
        id6=np.eye(6, dtype=np.float32),
    )

    # window offsets with in_bounds variant redirects (coord == 21)
    def offsets(xs_, ys_):
        offa = GW * xs_ + ys_
        ex = xs_ == 21
        ey = ys_ == 21
        off = np.where(ex & ey, OFF_D,
                       np.where(ex, OFF_B + ys_,
                                np.where(ey, OFF_C + GW * xs_, offa)))
        return off.astype(np.int32)

    in_maps = []
    for c in range(NCORES):
        sl = slice(BL * c, BL * (c + 1))
        xs_ = xc[sl].reshape(-1)
        ys_ = yc[sl].reshape(-1)
        in_maps.append(dict(
            xs=np.ascontiguousarray(x[sl].reshape(BL, C, H * W)),
            ps=np.ascontiguousarray(pred[sl].reshape(BL, 3, H * W)),
            offs=offsets(xs_, ys_)[None, :],
            embf=np.ascontiguousarray(
                emf[sl].reshape(UNITS, 4).T).reshape(1, -1).astype(np.float32),
            cont=np.ascontiguousarray(conf[sl].reshape(UNITS, 5).T),
            aamw=aamw.reshape(B, M, 6)[sl].reshape(UNITS, 6).astype(np.float32),
            aams=aams.reshape(B, M, P)[sl].reshape(UNITS, P).astype(np.float32),
            **consts,
        ))
    return in_maps


def kernel(**inputs):
    if "nc" not in _CACHE:
        _CACHE["nc"] = _build()
    nc = _CACHE["nc"]
    in_maps = _host_prep(inputs)
    res = run_bass_kernel_spmd(nc, in_maps, list(range(NCORES)),
                               **_CACHE.get("run_kwargs", {}))
    _CACHE["last_res"] = res
    wl = np.concatenate([r["o_wl"] for r in res.results], 0)   # (2048, 6)
    sl = np.concatenate([r["o_sl"] for r in res.results], 0)   # (2048, 225)
    wa = np.concatenate([r["o_wa"] for r in res.results], 0)
    sa = np.concatenate([r["o_sa"] for r in res.results], 0)
    wl = wl.reshape(B // 2, 2, M, 6)[:, None]
    sl = sl.reshape(B // 2, 2, M, P)[:, None]
    wa = wa.reshape(B // 2, 2, M, 1)[:, None].astype(np.int32)
    sa = sa.reshape(B // 2, 2, M, 1)[:, None].astype(np.int32)
    return wl, sl, wa, sa
